# revision 11
# baseline (speedup 1.0000x reference)
"""Trainium2 Bass kernel for nn_NPMaskChangeTransitionPrior.

Strategy (data-parallel over batches, 16 batches per core x 8 cores):
  Host: transpose x/embeddings to feature-major, fold the gumbel-sigmoid mask
  into the nn-MLP layer-0 weight, fold fc/nn output biases into the gs layer-0
  bias, pre-pack per-dim gs weights into 2-dims-per-128-partitions blocks,
  prescale the tangent layer-1 weight by W0's last row (so the JVP layer-0
  elementwise multiply becomes part of the matmul).

  Device per batch: fc-MLP (embeddings), nn-MLP over the raw time axis
  (computing H(t) once instead of per-window), assemble RHS[88, 2048] =
  [emb_h; H(w); H(w+1); H(w+2)]; then the gs stage: 4 dim-pairs x 4 window
  tiles of 512, primal in fp32, analytic tangent in bf16 with LeakyReLU
  derivative masks d = max(step(h), 0.2) as single tensor_scalar ops.
  Layer-3 matmuls of the 4 pairs are packed into one PSUM bank via
  tile_position col-groups. log|pdd| is accumulated with Ln+accum_out.

  Host post: add gs_b3, transpose residuals back, reduce the sld partials.
"""
import os
import numpy as np

import concourse.bass as bass
import concourse.bacc as bacc
import concourse.tile as tile
from concourse import mybir

F32 = mybir.dt.float32
BF16 = mybir.dt.bfloat16
AF = mybir.ActivationFunctionType
OP = mybir.AluOpType

T = 2050
NW = 2048
D = 8
H = 64
SLOPE = 0.2
TAU = 0.3
N_CORES = 8
NBC = 16          # batches per core
E_LO = T - H      # 1986: start of the tail tile (covers t=2048,2049)

WEIGHT_SPECS = [
    ("fcW0", [D, H], F32),
    ("fcW1blk", [128, 128], F32),
    ("fcW2blk", [128, 128], F32),
    ("fcb0p", [128, 1], F32),
    ("fcb1p", [128, 1], F32),
    ("nnW0eff", [D, H], F32),
    ("nnW1blk", [128, 128], F32),
    ("nnb0p", [128, 1], F32),
    ("nnb1p", [128, 1], F32),
    ("nnW2both", [128, 32], F32),
    ("gsL0", [4, 88, 128], F32),
    ("gsb0", [4, 128, 1], F32),
    ("gsW1", [4, 128, 128], F32),
    ("gsb1", [4, 128, 1], F32),
    ("gsW1t", [4, 128, 128], BF16),
    ("gsW2", [4, 128, 128], F32),
    ("gsb2", [4, 128, 1], F32),
    ("gsW2t", [4, 128, 128], BF16),
    ("gsW3", [4, 128, 32], F32),
    ("gsW3t", [4, 128, 32], BF16),
]


def build(nb):
    """Build the Bass program processing nb batches."""
    from contextlib import ExitStack

    nc = bacc.Bacc("TRN2")
    xT_d = nc.dram_tensor("xT", [nb, D, T], F32, kind="ExternalInput")
    embT_d = nc.dram_tensor("embT", [nb, D, NW], F32, kind="ExternalInput")
    wd = {}
    for name, shape, dt in WEIGHT_SPECS:
        wd[name] = nc.dram_tensor(name, shape, dt, kind="ExternalInput")
    resT_d = nc.dram_tensor("resT", [nb, D, NW], F32, kind="ExternalOutput")
    sldp_d = nc.dram_tensor("sldp", [128, nb * 4], F32, kind="ExternalOutput")

    with tile.TileContext(nc) as tc, ExitStack() as ctx:
        wp = ctx.enter_context(tc.tile_pool(name="w", bufs=1))
        io = ctx.enter_context(tc.tile_pool(name="io", bufs=2))
        ap_ = ctx.enter_context(tc.tile_pool(name="act", bufs=4))
        dp = ctx.enter_context(tc.tile_pool(name="dts", bufs=4))
        sp = ctx.enter_context(tc.tile_pool(name="stg", bufs=3))
        zp = ctx.enter_context(tc.tile_pool(name="psum", bufs=6, space="PSUM"))
        rp = ctx.enter_context(tc.tile_pool(name="psum_rp", bufs=2, space="PSUM"))

        # ---- load weights to SBUF
        w = {}
        for name, shape, dt in WEIGHT_SPECS:
            if shape[0] == 4:  # per-pair weights
                w[name] = []
                for j in range(4):
                    t_ = wp.tile(shape[1:], dt, tag=f"{name}{j}")
                    nc.sync.dma_start(out=t_, in_=wd[name][j])
                    w[name].append(t_)
            else:
                t_ = wp.tile(shape, dt, tag=name)
                nc.sync.dma_start(out=t_, in_=wd[name][:])
                w[name] = t_

        sld_t = wp.tile([128, nb * 4], F32, tag="sld")
        lnbias = wp.tile([128, 1], F32, tag="lnbias")
        nc.vector.memset(lnbias, 1e-30)

        def mm(out, lhsT, rhs, tp=None):
            nc.tensor.matmul(out, lhsT, rhs, start=True, stop=True,
                             tile_position=tp)

        def lrelu(out, z, bias):
            nc.scalar.activation(out, z, AF.Prelu, bias=bias, scale=1.0,
                                 alpha=SLOPE)

        def dmask(out, h_):
            nc.vector.tensor_scalar(out, h_, 0.0, SLOPE, OP.is_ge, OP.max)

        for b in range(nb):
            xt = io.tile([D, T], F32, tag="xt")
            nc.sync.dma_start(out=xt, in_=xT_d[b])
            et = io.tile([D, NW], F32, tag="et")
            nc.sync.dma_start(out=et, in_=embT_d[b])
            rhs = io.tile([88, NW], F32, tag="rhs")

            # ---------- fc MLP: embT -> emb_h rows of RHS (bias b2 folded out)
            for k in range(2):
                s0 = 1024 * k
                z0 = zp.tile([128, 512], F32, tag="z")
                mm(z0[0:64], w["fcW0"], et[:, s0:s0 + 512], tp=(0, 0))
                mm(z0[64:128], w["fcW0"], et[:, s0 + 512:s0 + 1024], tp=(0, 64))
                h0 = ap_.tile([128, 512], F32, tag="h")
                lrelu(h0, z0, w["fcb0p"])
                z1 = zp.tile([128, 512], F32, tag="z")
                mm(z1, w["fcW1blk"], h0)
                h1 = ap_.tile([128, 512], F32, tag="h")
                lrelu(h1, z1, w["fcb1p"])
                z2 = zp.tile([128, 512], F32, tag="z")
                mm(z2, w["fcW2blk"], h1)
                st = sp.tile([128, 512], F32, tag="stg")
                nc.vector.tensor_copy(st, z2)
                nc.sync.dma_start(out=rhs[0:64, s0:s0 + 512], in_=st[0:64])
                nc.sync.dma_start(out=rhs[0:64, s0 + 512:s0 + 1024],
                                  in_=st[64:128])

            # ---------- nn MLP over t -> H rows of RHS (3 shifted bands)
            bank = zp.tile([128, 512], F32, tag="z")
            for k in range(2):
                s0 = 1024 * k
                z0 = zp.tile([128, 512], F32, tag="z")
                mm(z0[0:64], w["nnW0eff"], xt[:, s0:s0 + 512], tp=(0, 0))
                mm(z0[64:128], w["nnW0eff"], xt[:, s0 + 512:s0 + 1024],
                   tp=(0, 64))
                h0 = ap_.tile([128, 512], F32, tag="h")
                lrelu(h0, z0, w["nnb0p"])
                z1 = zp.tile([128, 512], F32, tag="z")
                mm(z1, w["nnW1blk"], h0)
                h1 = ap_.tile([128, 512], F32, tag="h")
                lrelu(h1, z1, w["nnb1p"])
                c0, c1 = 2 * k, 2 * k + 1
                mm(bank[32 * c0:32 * c0 + 32], w["nnW2both"][0:64], h1[0:64],
                   tp=(0, 32 * c0))
                mm(bank[32 * c1:32 * c1 + 32], w["nnW2both"][64:128],
                   h1[64:128], tp=(64, 32 * c1))
            sb = sp.tile([128, 512], F32, tag="stg")
            nc.vector.tensor_copy(sb, bank)
            # tail tile: samples [E_LO, T) -> provides H[2048], H[2049]
            z0e = zp.tile([64, 64], F32, tag="z")
            mm(z0e, w["nnW0eff"], xt[:, E_LO:T])
            h0e = ap_.tile([64, 64], F32, tag="he")
            lrelu(h0e, z0e, w["nnb0p"][0:64])
            z1e = zp.tile([64, 64], F32, tag="z")
            mm(z1e, w["nnW1blk"][0:64, 0:64], h0e)
            h1e = ap_.tile([64, 64], F32, tag="he")
            lrelu(h1e, z1e, w["nnb1p"][0:64])
            bke = zp.tile([32, 64], F32, tag="z")
            mm(bke, w["nnW2both"][0:64], h1e, tp=(0, 0))
            se = sp.tile([32, 64], F32, tag="stge")
            nc.vector.tensor_copy(se, bke)
            # band copies into RHS rows 64..88: RHS[64+8l+d, w] = H[d, w+l]
            for l in range(3):
                r0 = 64 + 8 * l
                for c in range(4):
                    dlo = max(0, 512 * c - l)
                    dhi = min(NW, 512 * c + 512 - l)
                    slo = dlo - (512 * c - l)
                    nc.sync.dma_start(
                        out=rhs[r0:r0 + 8, dlo:dhi],
                        in_=sb[32 * c:32 * c + 8, slo:slo + (dhi - dlo)])
                if l > 0:
                    # columns NW-l .. NW-1 come from the tail tile
                    dlo = NW - l
                    slo = (dlo + l) - E_LO
                    nc.sync.dma_start(out=rhs[r0:r0 + 8, dlo:NW],
                                      in_=se[0:8, slo:slo + l])

            # ---------- gs stage
            for wt in range(4):
                w0 = 512 * wt
                resb = rp.tile([128, 512], F32, tag="rp")
                pddb = rp.tile([128, 512], F32, tag="rp")
                for j in range(4):
                    z0 = zp.tile([128, 512], F32, tag="z")
                    mm(z0, w["gsL0"][j], rhs[:, w0:w0 + 512])
                    h0 = ap_.tile([128, 512], F32, tag="h")
                    lrelu(h0, z0, w["gsb0"][j])
                    d0 = dp.tile([128, 512], BF16, tag="d")
                    dmask(d0, h0)
                    z1p = zp.tile([128, 512], F32, tag="z")
                    mm(z1p, w["gsW1"][j], h0)
                    z1t = zp.tile([128, 512], F32, tag="z")
                    mm(z1t, w["gsW1t"][j], d0)
                    h1 = ap_.tile([128, 512], F32, tag="h")
                    lrelu(h1, z1p, w["gsb1"][j])
                    d1 = dp.tile([128, 512], BF16, tag="d")
                    dmask(d1, h1)
                    t1 = dp.tile([128, 512], BF16, tag="t")
                    nc.vector.tensor_tensor(t1, d1, z1t, OP.mult)
                    z2p = zp.tile([128, 512], F32, tag="z")
                    mm(z2p, w["gsW2"][j], h1)
                    z2t = zp.tile([128, 512], F32, tag="z")
                    mm(z2t, w["gsW2t"][j], t1)
                    h2 = ap_.tile([128, 512], F32, tag="h")
                    lrelu(h2, z2p, w["gsb2"][j])
                    d2 = dp.tile([128, 512], BF16, tag="d")
                    dmask(d2, h2)
                    t2 = dp.tile([128, 512], BF16, tag="t")
                    nc.vector.tensor_tensor(t2, d2, z2t, OP.mult)
                    mm(resb[32 * j:32 * j + 32], w["gsW3"][j], h2,
                       tp=(0, 32 * j))
                    mm(pddb[32 * j:32 * j + 32], w["gsW3t"][j], t2,
                       tp=(0, 32 * j))
                # epilogue: residuals out + log|pdd| accumulation
                rs = sp.tile([128, 512], F32, tag="stg")
                nc.vector.tensor_copy(rs, resb)
                for j in range(4):
                    nc.sync.dma_start(
                        out=resT_d[b][2 * j:2 * j + 2, w0:w0 + 512],
                        in_=rs[32 * j:32 * j + 2, :])
                pq = sp.tile([128, 512], BF16, tag="pq")
                nc.scalar.activation(pq, pddb, AF.Square)
                dump = sp.tile([128, 512], BF16, tag="dump")
                nc.scalar.activation(dump, pq, AF.Ln, bias=lnbias,
                                     accum_out=sld_t[:, b * 4 + wt:b * 4 + wt + 1])

        nc.sync.dma_start(out=sldp_d[:], in_=sld_t)
    nc.compile()
    return nc


# ---------------------------------------------------------------------------
# host side

def host_pack(inputs):
    """Compute folded/packed weight arrays shared by all cores."""
    f32 = np.float32
    g = {k: np.asarray(v, f32) for k, v in inputs.items()}
    u = np.clip(g['u_noise'], 1e-6, 1 - 1e-6)
    mask = 1.0 / (1.0 + np.exp(-((np.log(u) - np.log1p(-u)) / TAU)))
    mask = (mask * (1.0 - np.eye(D, dtype=f32))).astype(f32)
    W0eff = (g['nn_W0'][:D] + mask @ g['nn_W0'][D:]).astype(f32)

    blk = lambda A, B: np.block(
        [[A, np.zeros((64, 64), f32)], [np.zeros((64, 64), f32), B]])

    wsrc = {}
    wsrc['fcW0'] = g['fc_W0']
    wsrc['fcW1blk'] = blk(g['fc_W1'], g['fc_W1'])
    wsrc['fcW2blk'] = blk(g['fc_W2'], g['fc_W2'])
    wsrc['fcb0p'] = np.tile(g['fc_b0'], 2)[:, None]
    wsrc['fcb1p'] = np.tile(g['fc_b1'], 2)[:, None]
    wsrc['nnW0eff'] = W0eff
    wsrc['nnW1blk'] = blk(g['nn_W1'], g['nn_W1'])
    wsrc['nnb0p'] = np.tile(g['nn_b0'], 2)[:, None]
    wsrc['nnb1p'] = np.tile(g['nn_b1'], 2)[:, None]
    w2pad = np.zeros((64, 32), f32)
    w2pad[:, 0:8] = g['nn_W2']
    wsrc['nnW2both'] = np.concatenate([w2pad, w2pad], axis=0)

    gs_W0, gs_W1, gs_W2, gs_W3 = g['gs_W0'], g['gs_W1'], g['gs_W2'], g['gs_W3']
    b0eff = np.zeros((D, 64), f32)
    for d in range(D):
        W = gs_W0[d]
        b0eff[d] = (g['gs_b0'][d] + W[0:64].T @ g['fc_b2']
                    + W[64:72].T @ g['nn_b2'] + W[72:80].T @ g['nn_b2']
                    + W[80] * g['nn_b2'][d])
    w0L = gs_W0[:, 80, :]
    What1 = gs_W1 * w0L[:, :, None]

    gsL0 = np.zeros((4, 88, 128), f32)
    gsb0 = np.zeros((4, 128, 1), f32)
    gsW1 = np.zeros((4, 128, 128), f32)
    gsb1 = np.zeros((4, 128, 1), f32)
    gsW1t = np.zeros((4, 128, 128), f32)
    gsW2 = np.zeros((4, 128, 128), f32)
    gsb2 = np.zeros((4, 128, 1), f32)
    gsW3 = np.zeros((4, 128, 32), f32)
    for j in range(4):
        for i, d in enumerate((2 * j, 2 * j + 1)):
            c = slice(64 * i, 64 * i + 64)
            gsL0[j, 0:80, c] = gs_W0[d][0:80]
            gsL0[j, 80 + d, c] = gs_W0[d][80]
            gsb0[j, c, 0] = b0eff[d]
            gsW1[j, c, c] = gs_W1[d]
            gsb1[j, c, 0] = g['gs_b1'][d]
            gsW1t[j, c, c] = What1[d]
            gsW2[j, c, c] = gs_W2[d]
            gsb2[j, c, 0] = g['gs_b2'][d]
            gsW3[j, c, i] = gs_W3[d, :, 0]
    wsrc.update(gsL0=gsL0, gsb0=gsb0, gsW1=gsW1, gsb1=gsb1, gsW2=gsW2,
                gsb2=gsb2, gsW3=gsW3)
    import ml_dtypes
    bf = lambda a: a.astype(ml_dtypes.bfloat16)
    wsrc['gsW1t'] = bf(gsW1t)
    wsrc['gsW2t'] = bf(gsW2)
    wsrc['gsW3t'] = bf(gsW3)
    return wsrc, mask


def host_inputs_per_core(inputs, nb=NBC, n_cores=N_CORES):
    f32 = np.float32
    x = np.asarray(inputs['x'], f32)
    emb = np.asarray(inputs['embeddings'], f32)
    B = x.shape[0]
    xT = np.ascontiguousarray(x.transpose(0, 2, 1))          # [B, 8, T]
    embT = np.ascontiguousarray(
        emb.reshape(B, NW, D).transpose(0, 2, 1))            # [B, 8, NW]
    wsrc, _ = host_pack(inputs)
    in_maps = []
    for c in range(n_cores):
        m = dict(wsrc)
        m['xT'] = np.ascontiguousarray(xT[c * nb:(c + 1) * nb])
        m['embT'] = np.ascontiguousarray(embT[c * nb:(c + 1) * nb])
        in_maps.append(m)
    return in_maps


def host_post(inputs, results, nb=NBC):
    f32 = np.float32
    gs_b3 = np.asarray(inputs['gs_b3'], f32)[:, 0]           # [8]
    n_cores = len(results)
    B = n_cores * nb
    residuals = np.zeros((B, NW, D), f32)
    sld = np.zeros((B,), f32)
    rows = np.array([32 * j + k for j in range(4) for k in range(2)])
    for c, r in enumerate(results):
        rt = np.asarray(r['resT'])                           # [nb, 8, NW]
        residuals[c * nb:(c + 1) * nb] = (
            rt.transpose(0, 2, 1) + gs_b3[None, None, :])
        sp_ = np.asarray(r['sldp'])[rows]                    # [8, nb*4]
        # device accumulates log(pdd^2) = 2*log|pdd|
        sld[c * nb:(c + 1) * nb] = (
            0.5 * sp_.sum(axis=0).reshape(nb, 4).sum(axis=1))
    return residuals, sld


_CACHED = {}


def kernel(**inputs):
    from concourse.bass_utils import run_bass_kernel_spmd

    if 'nc' not in _CACHED:
        _CACHED['nc'] = build(NBC)
    nc = _CACHED['nc']
    in_maps = host_inputs_per_core(inputs)
    trace = bool(int(os.environ.get('KERNEL_TRACE', '0')))
    out = run_bass_kernel_spmd(nc, in_maps, core_ids=list(range(N_CORES)),
                               trace=trace)
    _CACHED['last_results'] = out
    return host_post(inputs, out.results)
